# revision 37
# baseline (speedup 1.0000x reference)
"""Multi-head self-attention (B=4, L=2048, D=512, H=4, Hd=128) on 8 TRN2 cores.

Sharding: core c handles batch b = c//2 and head-pair p = c%2 (heads 2p, 2p+1).
Each core computes a partial output y_part[b] = sum_{h in pair} ctx_h @ Wo_h.T;
host gathers: y[b] = y_part[core 2b] + y_part[core 2b+1] + bo.

Dataflow per core (all matmuls bf16 inputs, fp32 PSUM accumulation):
  xT [512,2048] (host-pretransposed)  ->  QT,KT [hd,L] and V [L,hd] via PE
  scoresT [k,L_q] = KT_blk.T @ QT     (k-major: softmax along free dim never
  attnT = exp(scoresT/sqrt(hd))        needs a transpose anywhere)
  ctxT [hd,L_q] += V_blk.T @ attnT    (accumulate over k blocks)
  rowsum: DVE fold tree (T1 bf16 pairs -> T2 f32 -> S_a/S_b bf16), two
  accumulating rank-1 ones.T @ S matmuls (PE partition-reduce), cast r to
  bf16, PE rank-1 broadcast onesrow x r -> PSUM, reciprocal_approx_fast
  PSUM->SBUF gives rec128 with no DRAM bounce anywhere
  ctxT * rec128 in a single DVE op PSUM->SBUF bf16
  y_blk [L_q,D] += ctxT_blk.T @ WoT_h (accumulate over the 2 heads)
Scores go through single-bank PSUM tiles (6-buffer shared pool) with exp
split per bank, so the scores->exp->bank-recycle loop has ~3 groups of
slack; outproj windows are spread one-per-slot through the kk loops.
"""
import numpy as np
import ml_dtypes

B, L, D = 4, 2048, 512
H, HD = 4, 128
NCORES = 8
QW = 512          # query window (matmul N / PSUM bank)
NQC = L // QW     # 4 query windows
NKB = L // 128    # 16 key blocks
NDC = D // 128    # 4 contraction chunks for projections
SCALE = 1.0 / np.sqrt(HD)

_COMPILED = None


def _build():
    import concourse.bass as bass
    import concourse.mybir as mybir
    import concourse.tile as tile
    from concourse import bacc

    F32 = mybir.dt.float32
    BF16 = mybir.dt.bfloat16
    AF = mybir.ActivationFunctionType

    nc = bacc.Bacc("TRN2", target_bir_lowering=False, debug=False,
                   num_devices=NCORES)
    xT_d = nc.dram_tensor("xT", [D, L], BF16, kind="ExternalInput")
    # all weights host-packed into one tensor -> one DMA issue op:
    # [wq0,wk0,wq1,wk1 | wq2,wk2,wq3,wk3 | wv0..3 | wo0,wo1] along free dim
    wpk_d = nc.dram_tensor("wpk", [128, 16 * 256], BF16, kind="ExternalInput")
    bq_d = nc.dram_tensor("bq", [128, 2], F32, kind="ExternalInput")
    bk_d = nc.dram_tensor("bk", [128, 2], F32, kind="ExternalInput")
    bv_d = nc.dram_tensor("bv", [1, 256], F32, kind="ExternalInput")
    y_d = nc.dram_tensor("y", [L, D], F32, kind="ExternalOutput")

    with tile.TileContext(nc) as tc:
        with (
            tc.tile_pool(name="singles", bufs=1) as singles,
            tc.tile_pool(name="psu", bufs=6, space="PSUM") as psu,
            tc.tile_pool(name="psc", bufs=2, space="PSUM") as psc_pool,
            tc.tile_pool(name="attnp", bufs=16) as attnp,
            tc.tile_pool(name="f1p", bufs=5) as f1p,      # T1 bf16 [128,1024]
            tc.tile_pool(name="f2p", bufs=4) as f2p,      # T2 f32 [128,1024]
            tc.tile_pool(name="fsp", bufs=4) as fsp,      # S bf16 [128,512]
            tc.tile_pool(name="recp", bufs=4) as recp,
            tc.tile_pool(name="yp", bufs=5) as yp,
        ):
            # ---- constants on vector (fast semaphore path at start);
            # inputs: xt chunks first, alternating the two fast DMA queues ----
            warm_sb = singles.tile([128, 256], BF16)
            warmw_sb = singles.tile([128, 128], BF16)
            nc.vector.memset(warm_sb[:], 0.0)
            nc.vector.memset(warmw_sb[:], 0.0)
            ones_sb = singles.tile([128, 1], BF16)
            nc.vector.memset(ones_sb[:], 1.0)

            xt_sb = singles.tile([128, NDC, L], BF16)
            wpk_sb = singles.tile([128, 16 * 256], BF16)
            bq_sb = singles.tile([128, 2], F32)
            bk_sb = singles.tile([128, 2], F32)
            bv_sb = singles.tile([128, 256], F32)

            def wq(dc, h):
                return wpk_sb[:, 512 * dc + 128 * h:512 * dc + 128 * h + 128]

            def wk(dc, h):
                c0 = 512 * dc + 256
                return wpk_sb[:, c0 + 128 * h:c0 + 128 * h + 128]

            def wv(dc):
                return wpk_sb[:, 2048 + 256 * dc:2048 + 256 * dc + 256]

            def wo(h):
                return wpk_sb[:, 3072 + 512 * h:3072 + 512 * h + 512]

            # pass-A weights lead the scalar queue; xt arrives in
            # query-window column slices, window-0 slices first on every
            # queue, so the qc-ordered QK matmuls stream right behind DMA
            nc.scalar.dma_start(wpk_sb[:, 0:1024], wpk_d[:, 0:1024])
            xq = [nc.sync, nc.gpsimd, nc.scalar]
            for hf in range(2):
                cw = slice(1024 * hf, 1024 * hf + 1024)
                for c in range(NDC):
                    xq[(4 * hf + c) % 3].dma_start(
                        xt_sb[:, c, cw], xT_d[128 * c:128 * c + 128, cw])
            nc.scalar.dma_start(wpk_sb[:, 1024:4096], wpk_d[:, 1024:4096])
            nc.gpsimd.dma_start(bq_sb[:], bq_d[:])
            nc.gpsimd.dma_start(bk_sb[:], bk_d[:])
            nc.gpsimd.dma_start(
                bv_sb[:],
                bass.AP(tensor=bv_d.ap().tensor, offset=0, ap=[[0, 128], [1, 256]]))

            # PE warmup during the input-DMA window: short dummy matmuls
            # cycling the shared PSUM pool lift HAM to 8/8 before the burst.
            for wi in range(24):
                ps_w = psu.tile([128, 256], F32, name=f"ps_w{wi}", tag="u")
                nc.tensor.matmul(ps_w[:], warmw_sb[:], warm_sb[:],
                                 start=True, stop=True)

            # ---- Q/K projections, two passes over dc so pass A (dc 0,1)
            # overlaps the DMA of xt chunks 2,3; pass A partials park in SBUF
            # f32, pass B recombines with one fused DVE op per window ----
            qt_sb = singles.tile([128, 2, L], BF16)   # QT per head [hd, L]
            kt_sb = singles.tile([128, 2, L], BF16)
            v_sb = singles.tile([128, NKB, 256], BF16)  # V [k-part, kblk, 2*hd]
            qk_part = singles.tile([128, 16, QW], F32)

            add = mybir.AluOpType.add
            qk_jobs = [(h, p, qc) for qc in range(NQC) for h in range(2)
                       for p in range(2)]
            idx_of = {j: i for i, j in enumerate(qk_jobs)}
            for idx, (h, p, qc) in enumerate(qk_jobs):
                win = slice(QW * qc, QW * qc + QW)
                ps = psu.tile([128, QW], F32, name=f"ps_pa{idx}", tag="u")
                for dc in (0, 1):
                    nc.tensor.matmul(
                        ps[:], (wq, wk)[p](dc, h), xt_sb[:, dc, win],
                        start=(dc == 0), stop=(dc == 1))
                nc.scalar.activation(qk_part[:, idx, :], ps[:],
                                     AF.Copy, scale=1.0)
            # pass B ordered by xt half-chunk arrival (windows 0,1 need only
            # the first halves of xt2/xt3), then by when iteration 0/1 needs
            # each output: K h0 first within each half, Q qc2/3 last
            pb_jobs = ([(0, 1, 0), (0, 1, 1), (0, 0, 0), (0, 0, 1),
                        (1, 1, 0), (1, 1, 1), (1, 0, 0), (1, 0, 1)] +
                       [(0, 1, 2), (0, 1, 3), (1, 1, 2), (1, 1, 3),
                        (0, 0, 2), (0, 0, 3), (1, 0, 2), (1, 0, 3)])
            for (h, p, qc) in pb_jobs:
                idx = idx_of[(h, p, qc)]
                b_sb = (bq_sb, bk_sb)[p]
                o_sb = (qt_sb, kt_sb)[p]
                win = slice(QW * qc, QW * qc + QW)
                ps = psu.tile([128, QW], F32, name=f"ps_pb{idx}", tag="u")
                for dc in (2, 3):
                    nc.tensor.matmul(
                        ps[:], (wq, wk)[p](dc, h), xt_sb[:, dc, win],
                        start=(dc == 2), stop=(dc == 3))
                nc.vector.scalar_tensor_tensor(
                    o_sb[:, h, win], ps[:], b_sb[:, h:h + 1],
                    qk_part[:, idx, :], op0=add, op1=add)

            def emit_vproj(lb):
                ps = psu.tile([128, QW], F32, name=f"ps_v{lb}", tag="u")
                for dc in range(NDC):
                    nc.tensor.matmul(
                        ps[:, 0:256], xt_sb[:, dc, 128 * lb:128 * lb + 128],
                        wv(dc),
                        start=(dc == 0), stop=(dc == NDC - 1))
                nc.vector.tensor_add(v_sb[:, lb, :], ps[:, 0:256], bv_sb[:])

            # ---- attention: cross-iteration software pipeline ----
            ct_tiles = {}    # t -> [128, QW] bf16 normalized ctxT window
            rec_tiles = {}   # t -> [128, QW] f32 1/r broadcast
            s_tiles = {}     # t -> (S_a, S_b) bf16 folded exp partial sums

            def emit_scores_pair(t, qc, h, kk):
                win = slice(QW * qc, QW * qc + QW)
                at = attnp.tile([128, 1024], BF16, name=f"at{t}_{kk}",
                                tag="attn")
                k0 = 256 * kk
                for half in range(2):
                    ps_s = psu.tile([128, QW], F32,
                                    name=f"ps_s{t}_{kk}_{half}", tag="u")
                    nc.tensor.matmul(
                        ps_s[:], kt_sb[:, h, k0 + 128 * half:k0 + 128 * half + 128],
                        qt_sb[:, h, win], start=True, stop=True)
                    nc.scalar.activation(at[:, 512 * half:512 * half + 512],
                                         ps_s[:], AF.Exp, scale=SCALE)
                return at

            LAST_T = 2 * NQC - 1

            def emit_fold(t, kk, at_tiles, f1, f2):
                # DVE fold tree: T1 bf16 pair-adds, T2 f32, then S_a/S_b bf16
                # halves-folds feeding two accumulating rank-1 matmuls. For
                # the last iteration the tail groups skip the tree entirely
                # (direct rank-1s on the at tiles in the drain) so the
                # post-last-exp chain is pure PE.
                last = (t == LAST_T)
                if kk % 2 == 1 and not (last and kk == 7):
                    i = kk // 2
                    x1 = f1p.tile([128, 1024], BF16, name=f"t1_{t}_{i}",
                                  tag="f1")
                    nc.vector.tensor_add(x1[:], at_tiles[kk - 1][:],
                                         at_tiles[kk][:])
                    f1.append(x1)
                if kk == 3 or (kk == 7 and not last):
                    j = kk // 4
                    x2 = f2p.tile([128, 1024], F32, name=f"t2_{t}_{j}",
                                  tag="f2")
                    nc.vector.tensor_add(x2[:], f1[2 * j][:], f1[2 * j + 1][:])
                    s = fsp.tile([128, QW], BF16, name=f"s{t}_{j}", tag="fs")
                    nc.vector.tensor_add(s[:], x2[:, 0:512], x2[:, 512:1024])
                    if kk == 3:
                        s_tiles[t] = [s]
                    else:
                        s_tiles[t].append(s)
                if last and kk == 5:
                    sm = fsp.tile([128, QW], BF16, name=f"sm{t}", tag="fs")
                    nc.vector.tensor_add(sm[:], f1[2][:, 0:512],
                                         f1[2][:, 512:1024])
                    s_tiles[t].append(sm)

            def emit_rowsum_mm(t, at_tiles=None):
                # accumulating rank-1 partition-reduces, then the fast
                # reciprocal on the single [1,512] row straight out of PSUM
                movs = [s[:] for s in s_tiles[t]]
                if t == LAST_T:
                    movs += [at_tiles[6][:, 0:512], at_tiles[6][:, 512:1024],
                             at_tiles[7][:, 0:512], at_tiles[7][:, 512:1024]]
                ps_r = psu.tile([1, QW], F32, name=f"ps_r{t}", tag="u")
                for i, m in enumerate(movs):
                    nc.tensor.matmul(ps_r[:], ones_sb[:], m,
                                     start=(i == 0), stop=(i == len(movs) - 1))
                rb = recp.tile([1, QW], F32, name=f"rb{t}", tag="rb")
                nc.vector.reciprocal_approx_fast(rb[:], ps_r[:])
                return rb

            def emit_rowsum_bcast(t, rb):
                # gpsimd broadcast of 1/r across partitions: zero PE cost,
                # latency hidden by the half-iteration of slack before mul
                rec128 = recp.tile([128, QW], F32, name=f"rec128{t}",
                                   tag="rec128")
                nc.gpsimd.partition_broadcast(rec128[:], rb[:], channels=128)
                rec_tiles[t] = rec128

            def emit_ctx_pair(st, kk, ps_c):
                t, h, at_tiles = st
                hs = slice(128 * h, 128 * h + 128)
                at = at_tiles[kk]
                last = (kk == NKB // 2 - 1)
                nc.tensor.matmul(ps_c[:], v_sb[:, 2 * kk, hs], at[:, 0:512],
                                 start=(kk == 0), stop=False)
                nc.tensor.matmul(ps_c[:], v_sb[:, 2 * kk + 1, hs],
                                 at[:, 512:1024], start=False, stop=last)

            def finish_ctx(st, ps_c):
                t, h, at_tiles = st
                # single DVE op: normalize straight out of PSUM into SBUF bf16
                ct = singles.tile([128, QW], BF16, name=f"ct{t}")
                nc.vector.tensor_mul(ct[:], ps_c[:], rec_tiles[t][:])
                ct_tiles[t] = ct

            def emit_outproj1(qb, th0, th1, act_copy=False, dma_q=None):
                qq = qb % NQC
                qsl = slice(128 * qq, 128 * qq + 128)
                ps_y = psu.tile([128, D], F32, name=f"ps_y{qb}", tag="u")
                nc.tensor.matmul(ps_y[:], ct_tiles[th0][:, qsl],
                                 wo(0), start=True, stop=False)
                nc.tensor.matmul(ps_y[:], ct_tiles[th1][:, qsl],
                                 wo(1), start=False, stop=True)
                ysb = yp.tile([128, D], F32, name=f"ysb{qb}", tag="ysb")
                if act_copy:
                    nc.scalar.activation(ysb[:], ps_y[:], AF.Copy, scale=1.0)
                else:
                    nc.vector.tensor_copy(ysb[:], ps_y[:])
                (dma_q or nc.sync).dma_start(
                    y_d[128 * qb:128 * qb + 128, :], ysb[:])

            # outproj windows qb=0..11 spread one-per-slot into the kk loops;
            # qb 12..15 (head pair 6,7) can only run at the very end.
            owq = [(NQC * q + w, 2 * q, 2 * q + 1)
                   for q in range(3) for w in range(NQC)]

            def pop_outproj():
                if owq:
                    emit_outproj1(*owq.pop(0))

            prev = None
            for t in range(2 * NQC):
                qc, h = t // 2, t % 2
                ps_c = psc_pool.tile([128, QW], F32, name=f"ps_c{t}",
                                     tag="psc") if t > 0 else None
                at_tiles = []
                f1, f2 = [], []
                rb = None
                for kk in range(NKB // 2):
                    if prev is not None:
                        emit_ctx_pair(prev, kk, ps_c)
                        if kk == 2:
                            rb = emit_rowsum_mm(prev[0])
                        if kk == 4:
                            emit_rowsum_bcast(prev[0], rb)
                        if t >= 3 and kk in (3, 6):
                            pop_outproj()
                    elif kk < NKB // 4:
                        # iteration 0: V projection fills the exp-paced slack
                        emit_vproj(2 * kk)
                        emit_vproj(2 * kk + 1)
                    at_tiles.append(emit_scores_pair(t, qc, h, kk))
                    emit_fold(t, kk, at_tiles, f1, f2)
                if prev is None:
                    for lb in range(NKB // 2, NKB):
                        emit_vproj(lb)
                else:
                    finish_ctx(prev, ps_c)
                prev = (t, h, at_tiles)
            # drain: ctx of t=7 with the remaining qc=2 outproj windows and
            # the t=7 rowsum chain interleaved; no DRAM bounce anywhere so
            # the tail is a short PE/DVE chain.
            ps_c = psc_pool.tile([128, QW], F32, name="ps_c7", tag="psc")
            for kk in range(NKB // 2):
                emit_ctx_pair(prev, kk, ps_c)
                if kk == 2:
                    rb = emit_rowsum_mm(prev[0], prev[2])
                if kk == 4:
                    emit_rowsum_bcast(prev[0], rb)
                if kk in (3, 6):
                    pop_outproj()
            finish_ctx(prev, ps_c)
            for qq in range(NQC):
                emit_outproj1(NQC * 3 + qq, 6, 7, act_copy=(qq % 2 == 1),
                              dma_q=(nc.sync, nc.scalar)[qq % 2])

    nc.compile()
    return nc


def _get_compiled():
    global _COMPILED
    if _COMPILED is None:
        _COMPILED = _build()
    return _COMPILED


def make_in_maps(x, Wq, bq, Wk, bk, Wv, bv, Wo):
    bf16 = ml_dtypes.bfloat16
    xT = {b: np.ascontiguousarray(x[b].T).astype(bf16) for b in range(B)}
    WqT, WkT, WvT, WoT = (np.ascontiguousarray(W.T) for W in (Wq, Wk, Wv, Wo))
    in_maps = []
    for c in range(NCORES):
        b = c // 2
        p = c % 2
        hs = slice(256 * p, 256 * p + 256)
        wq_c, wk_c, wv_c = (W[:, hs] for W in (WqT, WkT, WvT))
        wo_c = WoT[hs, :]
        # packed weights [128, 16*256]:
        # [wq0,wk0,wq1,wk1 | wq2,wk2,wq3,wk3 | wv0..3 | wo0,wo1]
        blocks = []
        for dc in range(NDC):
            r = slice(128 * dc, 128 * dc + 128)
            blocks += [wq_c[r, :], wk_c[r, :]]
        for dc in range(NDC):
            blocks.append(wv_c[128 * dc:128 * dc + 128, :])
        for h in range(2):
            blocks.append(wo_c[128 * h:128 * h + 128, :])
        wpk = np.concatenate(blocks, axis=1).astype(bf16)
        in_maps.append({
            "xT": xT[b],
            "wpk": np.ascontiguousarray(wpk),
            "bq": np.ascontiguousarray(bq[hs].reshape(2, 128).T),
            "bk": np.ascontiguousarray(bk[hs].reshape(2, 128).T),
            "bv": bv[hs].reshape(1, 256).copy(),
        })
    return in_maps


def kernel(x, Wq, bq, Wk, bk, Wv, bv, Wo, bo):
    from concourse.bass_utils import run_bass_kernel_spmd

    x = np.asarray(x, np.float32)
    Wq, Wk, Wv, Wo = (np.asarray(w, np.float32) for w in (Wq, Wk, Wv, Wo))
    bq, bk, bv, bo = (np.asarray(b, np.float32) for b in (bq, bk, bv, bo))

    in_maps = make_in_maps(x, Wq, bq, Wk, bk, Wv, bv, Wo)
    nc = _get_compiled()
    try:
        res = run_bass_kernel_spmd(nc, in_maps, list(range(NCORES)))
    except Exception:
        # one retry: transient device wedges usually clear on re-execution
        res = run_bass_kernel_spmd(nc, in_maps, list(range(NCORES)))
    y = np.empty((B, L, D), np.float32)
    for b in range(B):
        y[b] = res.results[2 * b]["y"] + res.results[2 * b + 1]["y"] + bo
    return y
